# revision 37
# baseline (speedup 1.0000x reference)
"""Trainium2 Bass kernel for a pre-norm transformer block (attention + GELU MLP).

Problem shapes: x [4, 2048, 768], 12 heads x 64, MLP hidden 3072, fp32.

Sharding (8 cores, no collectives): core = (batch b = core//2, parity p = core%2).
Each batch's 16 row-tiles of 128 tokens are split by tile-index parity; a core
owns 8 row-tiles ("slots") and computes the complete block output for them.
K/V are computed locally from the full 2048-token context, so cores are fully
independent.  One SPMD program serves both parities: slot i always attends to
context tiles 0..2i+1, and a per-core 2x[128,128] multiplicative mask encodes
whether the trailing context tile is the causal diagonal (odd parity), or the
diagonal is one tile earlier and the trailing tile is junk (even parity).

v3 layout/scheduling notes (vs the 608us baseline):
  * K^T and Q^T are produced DIRECTLY by d-major matmuls (stationary = W
    d-chunk, moving = h^T over a 4-tile quad = 512 tokens), eliminating the
    per-tile K/Q transposes and the token-major bounce copies.
  * V carries 64 ones-columns, so the attention matmul broadcasts the softmax
    denominator across 64 PSUM partitions; normalization is one full-lane
    reciprocal + one tensor_tensor (the baseline burned ~3us per head on
    1-partition reciprocals, broadcast matmuls and PSUM->SBUF copies).
  * The head loop is software-pipelined (S(h) issues before attV(h-2)) and the
    V projection is deferred into the attention phase as tensor filler under
    the first heads' exps.  The PE array drops 2.4->1.2GHz when it idles, so
    tensor-queue continuity is the point; psS is triple-buffered so the
    scalar-engine exp stream (the real phase-B floor, ~92us) never stutters
    the tensor queue at a fine grain.
  * Phase C interleaves Wo, the LN2 transposes and the two MLP matmuls to
    keep the tensor queue dense; W1/W2 are bf16 (half the DMA of fp32).
  * All matmuls bf16 (fp8 was measured at 2.4e-2 rel err - over the gate).
"""

import os

import ml_dtypes
import numpy as np

import concourse.bass as bass
import concourse.bacc as bacc
import concourse.mybir as mybir
import concourse.tile as tile
from concourse.bass_utils import run_bass_kernel_spmd
from concourse.masks import make_identity

F32 = mybir.dt.float32
BF16 = mybir.dt.bfloat16
F8 = mybir.dt.float8e4
EXP = mybir.ActivationFunctionType.Exp
IDENT = mybir.ActivationFunctionType.Identity
GELU = mybir.ActivationFunctionType.Gelu
SQRT = mybir.ActivationFunctionType.Sqrt
MUL = mybir.AluOpType.mult
ADD = mybir.AluOpType.add

B, T, C, H, D = 4, 2048, 768, 12, 64
MH = 4 * C  # 3072
EPS = 1e-5
NT = T // 128  # 16 context tiles
NS = 8  # own slots per core
CB = C // 128  # 6 c-chunks (also 6 d-chunks = head pairs)
MB = MH // 128  # 24 mlp chunks
HP = H // 2  # 6 head pairs
CCHUNKS = ((0, 512), (512, 256))

# per-(head, ctx tile) score widths and offsets inside an expS row
SW = [(NS - j // 2) * 128 for j in range(NT)]
SOFF = [sum(SW[:j]) for j in range(NT)]
STOT = sum(SW)  # 9216


def _schunks(w):
    out, pos = [], 0
    while w > pos:
        out.append((pos, min(512, w - pos)))
        pos += 512
    return out


def _ln_stats(nc, pool, x_sb, eps_t):
    """Return (rstd, negms) [128,1] tiles: h = x*rstd + negms normalizes x."""
    xg = x_sb.rearrange("p (s f) -> p s f", f=256)
    stats = pool.tile([128, 3, 6], F32, tag="ln_stats", name="ln_stats")
    for s in range(3):
        nc.vector.bn_stats(out=stats[:, s, :], in_=xg[:, s, :])
    mv = pool.tile([128, 2], F32, tag="ln_mv", name="ln_mv")
    nc.vector.bn_aggr(out=mv[:], in_=stats[:])
    rstd = pool.tile([128, 1], F32, tag="ln_rstd", name="ln_rstd")
    nc.scalar.activation(out=rstd[:], in_=mv[:, 1:2], func=SQRT,
                         bias=eps_t[:], scale=1.0)
    nc.vector.reciprocal(out=rstd[:], in_=rstd[:])
    negms = pool.tile([128, 1], F32, tag="ln_negms", name="ln_negms")
    nc.vector.scalar_tensor_tensor(
        out=negms[:], in0=mv[:, 0:1], scalar=-1.0, in1=rstd[:],
        op0=MUL, op1=MUL)
    return rstd, negms


def build_program():
    nc = bacc.Bacc()
    x_ctx = nc.declare_dram_parameter("x_ctx", [NT, 128, C], F32, isOutput=False)
    x_own = nc.declare_dram_parameter("x_own", [NS, 128, C], F32, isOutput=False)
    # d-major QK weights: [c-part 128, c-chunk, d-chunk, 128]
    wq = nc.declare_dram_parameter("wq", [128, CB, CB, 128], BF16, isOutput=False)
    wk = nc.declare_dram_parameter("wk", [128, CB, CB, 128], BF16, isOutput=False)
    # token-major V weights: [c-part 128, c-chunk, 768]
    wv = nc.declare_dram_parameter("wv", [128, CB, C], BF16, isOutput=False)
    wo = nc.declare_dram_parameter("wo", [CB, 128, C], BF16, isOutput=False)
    w1 = nc.declare_dram_parameter("w1", [MB, 128, CB, 128], BF16, isOutput=False)
    w2 = nc.declare_dram_parameter("w2", [MB, 128, C], BF16, isOutput=False)
    mask = nc.declare_dram_parameter("mask", [128, 2, 128], BF16, isOutput=False)
    y = nc.declare_dram_parameter("y", [NS, 128, C], F32, isOutput=True)

    with tile.TileContext(nc) as tc:
        with (
            tc.tile_pool(name="singles", bufs=1) as singles,
            tc.tile_pool(name="small", bufs=4) as small,
            tc.tile_pool(name="norm", bufs=1) as normp,
            tc.tile_pool(name="persist", bufs=1) as pers,
        ):
            identb = singles.tile([128, 128], BF16)
            identf = singles.tile([128, 128], F32)
            make_identity(nc, identf)
            nc.vector.tensor_copy(out=identb[:], in_=identf[:])
            eps_t = singles.tile([128, 1], F32)
            nc.vector.memset(eps_t, EPS)
            mask_t = singles.tile([128, 2, 128], BF16)
            nc.sync.dma_start(out=mask_t[:], in_=mask[:])
            negln4 = singles.tile([128, 1], F32)
            nc.vector.memset(negln4, -1.3862943611198906)

            ATT = [pers.tile([128, NS * 128], BF16, tag=f"AT{a}",
                             name=f"AT{a}") for a in range(HP)]
            wot = [pers.tile([128, C], BF16, tag=f"wo{cb}", name=f"wo{cb}")
                   for cb in range(CB)]

            with tc.tile_pool(name="attn", bufs=1) as ap:
                # KT/QT: [d-part 128 (2 heads), d-chunk(pair), tokens]
                KT = ap.tile([128, CB, T], BF16, tag="KT", name="KT")
                QTe = ap.tile([128, CB, NS * 128], BF16, tag="QTe", name="QTe")
                QTo = ap.tile([128, CB, NS * 128], BF16, tag="QTo", name="QTo")
                nc.gpsimd.memset(QTe[D:128, :, :], 0.0)
                nc.gpsimd.memset(QTo[0:D, :, :], 0.0)
                # V + 64 ones-columns per head (denominator broadcast)
                VA = [ap.tile([128, H, 96], BF16, tag=f"VA{j}", name=f"VA{j}")
                      for j in range(NT)]
                for j in range(NT):
                    nc.gpsimd.memset(VA[j][:, :, D:96], 1.0)
                wv_s = ap.tile([128, CB, C], BF16, tag="wv", name="wv")
                nc.sync.dma_start(out=wv_s[:], in_=wv[:])

                # ---- Phase A+B fused: LN1; Q^T first; then per ctx quad:
                #      K^T, V, S/exp for heads 0,1 (keeps tensor queue dense);
                #      then the remaining heads' S/attV pipeline. ------------
                with (
                    tc.tile_pool(name="pB", bufs=2) as pB,
                ):
                    expS = [None] * H

                    with (
                        tc.tile_pool(name="pAx", bufs=3) as pAx,
                        tc.tile_pool(name="pAh", bufs=3) as pAh,
                        tc.tile_pool(name="pAw", bufs=1) as pAw,
                        tc.tile_pool(name="pHT", bufs=2) as pHT,
                        tc.tile_pool(name="psTr", bufs=2, space="PSUM") as psTr,
                        tc.tile_pool(name="psP", bufs=2, space="PSUM") as psP,
                        tc.tile_pool(name="psS", bufs=2, space="PSUM") as psS,
                    ):
                        def ln_tile(x_src, ht_dst):
                            xt = pAx.tile([128, C], F32, tag="xt", name="xt")
                            nc.sync.dma_start(out=xt[:], in_=x_src)
                            rstd, negms = _ln_stats(nc, small, xt, eps_t)
                            h = pAh.tile([128, C], BF16, tag="h", name="h")
                            nc.scalar.activation(out=h[:], in_=xt[:],
                                                 func=IDENT, bias=negms[:],
                                                 scale=rstd[:])
                            pt = psTr.tile([128, CB, 128], BF16, tag="tr",
                                           name="tr")
                            for cb in range(CB):
                                nc.tensor.transpose(
                                    pt[:, cb, :],
                                    h[:, cb * 128:(cb + 1) * 128], identb[:])
                            nc.scalar.copy(out=ht_dst, in_=pt[:])

                        def qk_quad(w_sb, hT, n, dsts, col0):
                            for dc in range(CB):
                                pq = psP.tile([128, 512], F32, tag="qk",
                                              name="qk")
                                for cb in range(CB):
                                    nc.tensor.matmul(
                                        pq[:, 0:n * 128], w_sb[:, cb, dc, :],
                                        hT[:, cb, 0:n, :],
                                        start=(cb == 0), stop=(cb == CB - 1))
                                cols = slice(col0, col0 + n * 128)
                                if len(dsts) == 1:
                                    nc.vector.tensor_copy(
                                        out=dsts[0][:, dc, cols],
                                        in_=pq[:, 0:n * 128])
                                else:
                                    nc.vector.tensor_copy(
                                        out=dsts[0][0:D, dc, cols],
                                        in_=pq[0:D, 0:n * 128])
                                    nc.scalar.copy(
                                        out=dsts[1][D:128, dc, cols],
                                        in_=pq[D:128, 0:n * 128])

                        def s_part(h, jlist):
                            a = h // 2
                            qt = QTe if h % 2 == 0 else QTo
                            es = expS[h]
                            for j in jlist:
                                st = psS.tile([128, 1024], F32, tag="S",
                                              name="S")
                                for (c0, cw) in _schunks(SW[j]):
                                    q0 = (j // 2) * 128 + c0
                                    nc.tensor.matmul(
                                        st[:, c0:c0 + cw],
                                        KT[:, a, j * 128:(j + 1) * 128],
                                        qt[:, a, q0:q0 + cw],
                                        start=True, stop=True)
                                nc.scalar.activation(
                                    out=es[:, SOFF[j]:SOFF[j] + SW[j]],
                                    in_=st[:, 0:SW[j]],
                                    func=EXP, scale=float(D) ** -0.5)
                                nc.vector.tensor_tensor(
                                    out=es[:, SOFF[j]:SOFF[j] + 128],
                                    in0=es[:, SOFF[j]:SOFF[j] + 128],
                                    in1=mask_t[:, j % 2, :], op=MUL)

                        def v_tile(hT, jj, j):
                            pV = psS.tile([128, 1024], F32, tag="S", name="S")
                            for (n0, nw) in CCHUNKS:
                                for cb in range(CB):
                                    nc.tensor.matmul(
                                        pV[:, n0:n0 + nw], hT[:, cb, jj, :],
                                        wv_s[:, cb, n0:n0 + nw],
                                        start=(cb == 0), stop=(cb == CB - 1))
                            nc.scalar.copy(
                                out=VA[j][:, :, 0:D],
                                in_=pV[:, 0:C].rearrange("p (h d) -> p h d",
                                                         d=D))

                        # own tiles first: Q^T (wq shares the pAw buffer)
                        with (
                            tc.tile_pool(name="pHo", bufs=2) as pHo,
                        ):
                            wq_s = pAw.tile([128, CB, CB, 128], BF16,
                                            tag="wk", name="wq")
                            nc.sync.dma_start(out=wq_s[:], in_=wq[:])
                            for q in range(2):
                                hTo = pHo.tile([128, CB, 4, 128], BF16,
                                               tag="hTo", name="hTo")
                                for i in range(4):
                                    ln_tile(x_own[4 * q + i],
                                            hTo[:, :, i, :])
                                qk_quad(wq_s, hTo, 4, (QTe, QTo),
                                        4 * q * 128)

                        wk_s = pAw.tile([128, CB, CB, 128], BF16, tag="wk",
                                        name="wk")
                        nc.sync.dma_start(out=wk_s[:], in_=wk[:])
                        for hh in range(2):
                            expS[hh] = pB.tile([128, STOT], BF16, tag="eS",
                                               name=f"eS{hh}")
                        for q in range(4):
                            jq = list(range(4 * q, 4 * q + 4))
                            HTq = pHT.tile([128, CB, 4, 128], BF16,
                                           tag="HTq", name="HTq")
                            for jj in range(4):
                                ln_tile(x_ctx[jq[jj]], HTq[:, :, jj, :])
                            qk_quad(wk_s, HTq, 4, (KT,), 4 * q * 128)
                            for jj in range(4):
                                v_tile(HTq, jj, jq[jj])
                            s_part(0, jq)
                            s_part(1, jq)

                    for cb in range(CB):
                        nc.sync.dma_start(out=wot[cb][:], in_=wo[cb])

                    # remaining heads: software-pipelined S/attV
                    with (
                        tc.tile_pool(name="psS2", bufs=3, space="PSUM") as psS2,
                        tc.tile_pool(name="psA", bufs=2, space="PSUM") as psA,
                    ):
                        def s_head(h):
                            a = h // 2
                            qt = QTe if h % 2 == 0 else QTo
                            es = pB.tile([128, STOT], BF16, tag="eS",
                                         name=f"eS{h}")
                            expS[h] = es
                            for j in range(NT):
                                st = psS2.tile([128, 1024], F32, tag="S",
                                               name="S")
                                for (c0, cw) in _schunks(SW[j]):
                                    q0 = (j // 2) * 128 + c0
                                    nc.tensor.matmul(
                                        st[:, c0:c0 + cw],
                                        KT[:, a, j * 128:(j + 1) * 128],
                                        qt[:, a, q0:q0 + cw],
                                        start=True, stop=True)
                                nc.scalar.activation(
                                    out=es[:, SOFF[j]:SOFF[j] + SW[j]],
                                    in_=st[:, 0:SW[j]],
                                    func=EXP, scale=float(D) ** -0.5)
                                nc.vector.tensor_tensor(
                                    out=es[:, SOFF[j]:SOFF[j] + 128],
                                    in0=es[:, SOFF[j]:SOFF[j] + 128],
                                    in1=mask_t[:, j % 2, :], op=MUL)

                        def av_head(h):
                            a, z = h // 2, (h % 2) * D
                            for k in range(2):
                                at = psA.tile([128, 512], F32, tag="attn",
                                              name="attn")
                                js = range(8) if k == 0 else range(NT)
                                for j in js:
                                    i0 = j // 2
                                    lo = max(i0, 4 * k)
                                    ps = (lo - 4 * k) * 128
                                    w = (4 * k + 4 - lo) * 128
                                    rs = (lo - i0) * 128
                                    nc.tensor.matmul(
                                        at[0:96, ps:ps + w], VA[j][:, h, :],
                                        expS[h][:, SOFF[j] + rs:
                                                SOFF[j] + rs + w],
                                        start=(j == js[0]),
                                        stop=(j == js[-1]))
                                den = normp.tile([32, 512], F32, tag="den",
                                                 name="den")
                                nc.vector.tensor_copy(out=den[:],
                                                      in_=at[D:D + 32, :])
                                rcp = normp.tile([32, 512], F32, tag="rcp",
                                                 name="rcp")
                                nc.vector.reciprocal_approx_fast(
                                    out=rcp[:], in_=den[:])
                                ks = slice(k * 512, (k + 1) * 512)
                                nc.vector.tensor_tensor(
                                    out=ATT[a][z:z + 32, ks],
                                    in0=at[0:32, :], in1=rcp[:], op=MUL)
                                nc.vector.tensor_tensor(
                                    out=ATT[a][z + 32:z + D, ks],
                                    in0=at[32:D, :], in1=rcp[:], op=MUL)

                        av_head(0)
                        for h in range(2, H):
                            s_head(h)
                            av_head(h - 1)
                        av_head(H - 1)

            # ---- Phase C: Wo + residual; LN2 + MLP + residual ---------------
            with tc.tile_pool(name="p3w", bufs=1) as p3w:
                X2 = [p3w.tile([128, C], F32, tag=f"X2{i}", name=f"X2{i}")
                      for i in range(NS)]
                xbuf = [p3w.tile([128, C], F32, tag=f"xb{i}", name=f"xb{i}")
                        for i in range(NS)]
                for i in range(NS):
                    nc.sync.dma_start(out=xbuf[i][:], in_=x_own[i])
                W2S = p3w.tile([128, MB, C], BF16, tag="W2S", name="W2S")
                for m in range(MB):
                    nc.gpsimd.dma_start(out=W2S[:, m, :], in_=w2[m])

                h2T = [p3w.tile([128, NS * 128], BF16, tag=f"h2T{cb}",
                                name=f"h2T{cb}") for cb in range(CB)]
                hidT = [p3w.tile([128, NS * 128], BF16, tag=f"hid{m}",
                                 name=f"hid{m}") for m in range(MB)]

                with (
                    tc.tile_pool(name="pC", bufs=3) as pC,
                    tc.tile_pool(name="psW", bufs=2, space="PSUM") as psW,
                    tc.tile_pool(name="psT2", bufs=2, space="PSUM") as psT2,
                    tc.tile_pool(name="psM", bufs=2, space="PSUM") as psM,
                ):
                    h2buf = [None] * NS

                    def wo_slot(i):
                        """Wo matmul + residual + LN2; h2 kept for transposes."""
                        xt = xbuf[i]
                        pt = psW.tile([128, C], F32, tag="wops", name="wops")
                        for (n0, nw) in CCHUNKS:
                            for a in range(HP):
                                nc.tensor.matmul(
                                    pt[:, n0:n0 + nw],
                                    ATT[a][:, i * 128:(i + 1) * 128],
                                    wot[a][:, n0:n0 + nw],
                                    start=(a == 0), stop=(a == HP - 1))
                        nc.vector.tensor_tensor(
                            out=X2[i][:], in0=pt[:], in1=xt[:], op=ADD)
                        rstd, negms = _ln_stats(nc, small, X2[i], eps_t)
                        h2 = pC.tile([128, C], BF16, tag="h2", name=f"h2_{i}")
                        nc.scalar.activation(out=h2[:], in_=X2[i][:],
                                             func=IDENT, bias=negms[:],
                                             scale=rstd[:])
                        h2buf[i] = h2

                    def h2_tr(i):
                        pt = psT2.tile([128, CB, 128], BF16, tag="tr2",
                                       name="tr2")
                        for cb in range(CB):
                            nc.tensor.transpose(
                                pt[:, cb, :],
                                h2buf[i][:, cb * 128:(cb + 1) * 128], identb[:])
                        for cb in range(CB):
                            nc.vector.tensor_copy(
                                out=h2T[cb][:, i * 128:(i + 1) * 128],
                                in_=pt[:, cb, :])

                    def mlp1(m):
                        w1t = pC.tile([128, CB, 128], BF16, tag="w1t",
                                      name="w1t")
                        nc.sync.dma_start(out=w1t[:], in_=w1[m])
                        for sc in range(2):
                            pt = psM.tile([128, 512], F32, tag="mlp1",
                                          name="mlp1")
                            for cb in range(CB):
                                nc.tensor.matmul(
                                    pt[:], w1t[:, cb, :],
                                    h2T[cb][:, sc * 512:(sc + 1) * 512],
                                    start=(cb == 0), stop=(cb == CB - 1))
                            nc.scalar.activation(
                                out=hidT[m][:, sc * 512:(sc + 1) * 512],
                                in_=pt[:], func=GELU)

                    wo_slot(0)
                    wo_slot(1)
                    for i in range(2, NS):
                        wo_slot(i)
                        h2_tr(i - 2)
                    h2_tr(NS - 2)
                    h2_tr(NS - 1)
                    for m in range(MB):
                        mlp1(m)

                with (
                    tc.tile_pool(name="pD", bufs=2) as pD,
                    tc.tile_pool(name="psY", bufs=2, space="PSUM") as psY,
                ):
                    for i in range(NS):
                        yt = pD.tile([128, C], F32, tag="yt", name="yt")
                        pt = psY.tile([128, C], F32, tag="mlp2", name="mlp2")
                        for (n0, nw) in CCHUNKS:
                            for m in range(MB):
                                nc.tensor.matmul(
                                    pt[:, n0:n0 + nw],
                                    hidT[m][:, i * 128:(i + 1) * 128],
                                    W2S[:, m, n0:n0 + nw],
                                    start=(m == 0), stop=(m == MB - 1))
                        nc.vector.tensor_tensor(
                            out=yt[:], in0=pt[:], in1=X2[i][:], op=ADD)
                        nc.sync.dma_start(out=y[i], in_=yt[:])

    nc.finalize()
    return nc


_NC = None
LAST_RESULTS = None


def _get_program():
    global _NC
    if _NC is None:
        _NC = build_program()
    return _NC


def _core_inputs(inputs):
    x = np.ascontiguousarray(np.asarray(inputs["x"], np.float32))
    wq_f = np.transpose(np.asarray(inputs["Wq"], np.float32), (1, 0, 2)
                        ).reshape(C, C)
    wk_f = np.transpose(np.asarray(inputs["Wk"], np.float32), (1, 0, 2)
                        ).reshape(C, C)
    wv_f = np.transpose(np.asarray(inputs["Wv"], np.float32), (1, 0, 2)
                        ).reshape(C, C)
    # d-major: [c-part, c-chunk, d-chunk, 128]
    wq = np.ascontiguousarray(
        wq_f.reshape(CB, 128, CB, 128).transpose(1, 0, 2, 3)
    ).astype(ml_dtypes.bfloat16)
    wk = np.ascontiguousarray(
        wk_f.reshape(CB, 128, CB, 128).transpose(1, 0, 2, 3)
    ).astype(ml_dtypes.bfloat16)
    # token-major: [c-part, c-chunk, 768]
    wv = np.ascontiguousarray(
        wv_f.reshape(CB, 128, C).transpose(1, 0, 2)).astype(ml_dtypes.bfloat16)
    wo = np.asarray(inputs["Wo"], np.float32).reshape(CB, 128, C).astype(
        ml_dtypes.bfloat16)
    w1 = np.ascontiguousarray(
        np.asarray(inputs["W1"], np.float32).reshape(CB, 128, MB, 128)
        .transpose(2, 1, 0, 3)).astype(ml_dtypes.bfloat16)
    w2 = np.asarray(inputs["W2"], np.float32).reshape(MB, 128, C).astype(
        ml_dtypes.bfloat16)

    tri = (np.arange(128)[:, None] <= np.arange(128)[None, :]).astype(np.float32)
    masks = {
        0: np.stack([tri, np.zeros((128, 128), np.float32)], axis=1),
        1: np.stack([np.ones((128, 128), np.float32), tri], axis=1),
    }
    in_maps = []
    for core in range(8):
        b, p = core // 2, core % 2
        own = [2 * i + p for i in range(NS)]
        x_b = x[b].reshape(NT, 128, C)
        in_maps.append({
            "x_ctx": x_b,
            "x_own": np.ascontiguousarray(x_b[own]),
            "wq": wq, "wk": wk, "wv": wv, "wo": wo, "w1": w1, "w2": w2,
            "mask": np.ascontiguousarray(masks[p]).astype(ml_dtypes.bfloat16),
        })
    return in_maps


def kernel(**inputs):
    global LAST_RESULTS
    nc = _get_program()
    in_maps = _core_inputs(inputs)
    trace = bool(int(os.environ.get("KERNEL_TRACE", "0")))
    res = run_bass_kernel_spmd(
        nc, in_maps, core_ids=list(range(8)), trace=trace,
        trace_cores=list(range(8)) if trace else None,
    )
    LAST_RESULTS = res
    out = np.empty((B, T, C), np.float32)
    for core in range(8):
        b, p = core // 2, core % 2
        yc = res.results[core]["y"]  # [8, 128, 768]
        for i in range(NS):
            g = 2 * i + p
            out[b, g * 128:(g + 1) * 128, :] = yc[i]
    return out


# revision 38
# speedup vs baseline: 1.0290x; 1.0290x over previous
"""Trainium2 Bass kernel for a pre-norm transformer block (attention + GELU MLP).

Problem shapes: x [4, 2048, 768], 12 heads x 64, MLP hidden 3072, fp32.

Sharding (8 cores, no collectives): core = (batch b = core//2, parity p = core%2).
Each batch's 16 row-tiles of 128 tokens are split by tile-index parity; a core
owns 8 row-tiles ("slots") and computes the complete block output for them.
K/V are computed locally from the full 2048-token context, so cores are fully
independent.  One SPMD program serves both parities: slot i always attends to
context tiles 0..2i+1, and a per-core 2x[128,128] multiplicative mask encodes
whether the trailing context tile is the causal diagonal (odd parity), or the
diagonal is one tile earlier and the trailing tile is junk (even parity).

v3 layout/scheduling notes (vs the 608us baseline):
  * K^T and Q^T are produced DIRECTLY by d-major matmuls (stationary = W
    d-chunk, moving = h^T over a 4-tile quad = 512 tokens), eliminating the
    per-tile K/Q transposes and the token-major bounce copies.
  * V carries 64 ones-columns, so the attention matmul broadcasts the softmax
    denominator across 64 PSUM partitions; normalization is one full-lane
    reciprocal + one tensor_tensor (the baseline burned ~3us per head on
    1-partition reciprocals, broadcast matmuls and PSUM->SBUF copies).
  * The head loop is software-pipelined (S(h) issues before attV(h-2)) and the
    V projection is deferred into the attention phase as tensor filler under
    the first heads' exps.  The PE array drops 2.4->1.2GHz when it idles, so
    tensor-queue continuity is the point; psS is triple-buffered so the
    scalar-engine exp stream (the real phase-B floor, ~92us) never stutters
    the tensor queue at a fine grain.
  * Phase C interleaves Wo, the LN2 transposes and the two MLP matmuls to
    keep the tensor queue dense; W1/W2 are bf16 (half the DMA of fp32).
  * All matmuls bf16 (fp8 was measured at 2.4e-2 rel err - over the gate).
"""

import os

import ml_dtypes
import numpy as np

import concourse.bass as bass
import concourse.bacc as bacc
import concourse.mybir as mybir
import concourse.tile as tile
from concourse.bass_utils import run_bass_kernel_spmd
from concourse.masks import make_identity

F32 = mybir.dt.float32
BF16 = mybir.dt.bfloat16
F8 = mybir.dt.float8e4
EXP = mybir.ActivationFunctionType.Exp
IDENT = mybir.ActivationFunctionType.Identity
GELU = mybir.ActivationFunctionType.Gelu
SQRT = mybir.ActivationFunctionType.Sqrt
MUL = mybir.AluOpType.mult
ADD = mybir.AluOpType.add

B, T, C, H, D = 4, 2048, 768, 12, 64
MH = 4 * C  # 3072
EPS = 1e-5
NT = T // 128  # 16 context tiles
NS = 8  # own slots per core
CB = C // 128  # 6 c-chunks (also 6 d-chunks = head pairs)
MB = MH // 128  # 24 mlp chunks
HP = H // 2  # 6 head pairs
CCHUNKS = ((0, 512), (512, 256))

# per-(head, ctx tile) score widths and offsets inside an expS row
SW = [(NS - j // 2) * 128 for j in range(NT)]
SOFF = [sum(SW[:j]) for j in range(NT)]
STOT = sum(SW)  # 9216


def _schunks(w):
    out, pos = [], 0
    while w > pos:
        out.append((pos, min(512, w - pos)))
        pos += 512
    return out


def _ln_stats(nc, pool, x_sb, eps_t):
    """Return (rstd, negms) [128,1] tiles: h = x*rstd + negms normalizes x."""
    xg = x_sb.rearrange("p (s f) -> p s f", f=256)
    stats = pool.tile([128, 3, 6], F32, tag="ln_stats", name="ln_stats")
    for s in range(3):
        nc.vector.bn_stats(out=stats[:, s, :], in_=xg[:, s, :])
    mv = pool.tile([128, 2], F32, tag="ln_mv", name="ln_mv")
    nc.vector.bn_aggr(out=mv[:], in_=stats[:])
    rstd = pool.tile([128, 1], F32, tag="ln_rstd", name="ln_rstd")
    nc.scalar.activation(out=rstd[:], in_=mv[:, 1:2], func=SQRT,
                         bias=eps_t[:], scale=1.0)
    nc.vector.reciprocal(out=rstd[:], in_=rstd[:])
    negms = pool.tile([128, 1], F32, tag="ln_negms", name="ln_negms")
    nc.vector.scalar_tensor_tensor(
        out=negms[:], in0=mv[:, 0:1], scalar=-1.0, in1=rstd[:],
        op0=MUL, op1=MUL)
    return rstd, negms


def build_program():
    nc = bacc.Bacc()
    x_ctx = nc.declare_dram_parameter("x_ctx", [NT, 128, C], F32, isOutput=False)
    x_own = nc.declare_dram_parameter("x_own", [NS, 128, C], F32, isOutput=False)
    # d-major QK weights: [c-part 128, c-chunk, d-chunk, 128]
    wq = nc.declare_dram_parameter("wq", [128, CB, CB, 128], BF16, isOutput=False)
    wk = nc.declare_dram_parameter("wk", [128, CB, CB, 128], BF16, isOutput=False)
    # token-major V weights: [c-part 128, c-chunk, 768]
    wv = nc.declare_dram_parameter("wv", [128, CB, C], BF16, isOutput=False)
    wo = nc.declare_dram_parameter("wo", [CB, 128, C], BF16, isOutput=False)
    w1 = nc.declare_dram_parameter("w1", [MB, 128, CB, 128], BF16, isOutput=False)
    w2 = nc.declare_dram_parameter("w2", [MB, 128, C], BF16, isOutput=False)
    mask = nc.declare_dram_parameter("mask", [128, 2, 128], BF16, isOutput=False)
    y = nc.declare_dram_parameter("y", [NS, 128, C], F32, isOutput=True)

    with tile.TileContext(nc) as tc:
        with (
            tc.tile_pool(name="singles", bufs=1) as singles,
            tc.tile_pool(name="small", bufs=4) as small,
            tc.tile_pool(name="norm", bufs=1) as normp,
            tc.tile_pool(name="persist", bufs=1) as pers,
        ):
            identb = singles.tile([128, 128], BF16)
            identf = singles.tile([128, 128], F32)
            make_identity(nc, identf)
            nc.vector.tensor_copy(out=identb[:], in_=identf[:])
            eps_t = singles.tile([128, 1], F32)
            nc.vector.memset(eps_t, EPS)
            mask_t = singles.tile([128, 2, 128], BF16)
            nc.sync.dma_start(out=mask_t[:], in_=mask[:])
            negln4 = singles.tile([128, 1], F32)
            nc.vector.memset(negln4, -1.3862943611198906)

            ATT = [pers.tile([128, NS * 128], BF16, tag=f"AT{a}",
                             name=f"AT{a}") for a in range(HP)]
            wot = [pers.tile([128, C], BF16, tag=f"wo{cb}", name=f"wo{cb}")
                   for cb in range(CB)]

            with tc.tile_pool(name="attn", bufs=1) as ap:
                # KT/QT: [d-part 128 (2 heads), d-chunk(pair), tokens]
                KT = ap.tile([128, CB, T], BF16, tag="KT", name="KT")
                QTe = ap.tile([128, CB, NS * 128], BF16, tag="QTe", name="QTe")
                QTo = ap.tile([128, CB, NS * 128], BF16, tag="QTo", name="QTo")
                nc.gpsimd.memset(QTe[D:128, :, :], 0.0)
                nc.gpsimd.memset(QTo[0:D, :, :], 0.0)
                # V + 64 ones-columns per head (denominator broadcast)
                VA = [ap.tile([128, H, 96], BF16, tag=f"VA{j}", name=f"VA{j}")
                      for j in range(NT)]
                for j in range(NT):
                    nc.gpsimd.memset(VA[j][:, :, D:96], 1.0)
                wv_s = ap.tile([128, CB, C], BF16, tag="wv", name="wv")
                nc.sync.dma_start(out=wv_s[:], in_=wv[:])

                # ---- Phase A+B fused: LN1; Q^T first; then per ctx quad:
                #      K^T, V, S/exp for heads 0,1 (keeps tensor queue dense);
                #      then the remaining heads' S/attV pipeline. ------------
                with (
                    tc.tile_pool(name="pB", bufs=2) as pB,
                ):
                    expS = [None] * H

                    with (
                        tc.tile_pool(name="pAx", bufs=3) as pAx,
                        tc.tile_pool(name="pAh", bufs=3) as pAh,
                        tc.tile_pool(name="pAw", bufs=1) as pAw,
                        tc.tile_pool(name="pHT", bufs=2) as pHT,
                        tc.tile_pool(name="psTr", bufs=2, space="PSUM") as psTr,
                        tc.tile_pool(name="psP", bufs=2, space="PSUM") as psP,
                        tc.tile_pool(name="psS", bufs=2, space="PSUM") as psS,
                    ):
                        def ln_tile(x_src, ht_dst):
                            xt = pAx.tile([128, C], F32, tag="xt", name="xt")
                            nc.sync.dma_start(out=xt[:], in_=x_src)
                            rstd, negms = _ln_stats(nc, small, xt, eps_t)
                            h = pAh.tile([128, C], BF16, tag="h", name="h")
                            nc.scalar.activation(out=h[:], in_=xt[:],
                                                 func=IDENT, bias=negms[:],
                                                 scale=rstd[:])
                            pt = psTr.tile([128, CB, 128], BF16, tag="tr",
                                           name="tr")
                            for cb in range(CB):
                                nc.tensor.transpose(
                                    pt[:, cb, :],
                                    h[:, cb * 128:(cb + 1) * 128], identb[:])
                            nc.scalar.copy(out=ht_dst, in_=pt[:])

                        def qk_quad(w_sb, hT, n, dsts, col0):
                            for dc in range(CB):
                                pq = psP.tile([128, 512], F32, tag="qk",
                                              name="qk")
                                for cb in range(CB):
                                    nc.tensor.matmul(
                                        pq[:, 0:n * 128], w_sb[:, cb, dc, :],
                                        hT[:, cb, 0:n, :],
                                        start=(cb == 0), stop=(cb == CB - 1))
                                cols = slice(col0, col0 + n * 128)
                                if len(dsts) == 1:
                                    nc.vector.tensor_copy(
                                        out=dsts[0][:, dc, cols],
                                        in_=pq[:, 0:n * 128])
                                else:
                                    nc.vector.tensor_copy(
                                        out=dsts[0][0:D, dc, cols],
                                        in_=pq[0:D, 0:n * 128])
                                    nc.scalar.copy(
                                        out=dsts[1][D:128, dc, cols],
                                        in_=pq[D:128, 0:n * 128])

                        def s_part(h, jlist):
                            a = h // 2
                            qt = QTe if h % 2 == 0 else QTo
                            es = expS[h]
                            for j in jlist:
                                st = psS.tile([128, 1024], F32, tag="S",
                                              name="S")
                                for (c0, cw) in _schunks(SW[j]):
                                    q0 = (j // 2) * 128 + c0
                                    nc.tensor.matmul(
                                        st[:, c0:c0 + cw],
                                        KT[:, a, j * 128:(j + 1) * 128],
                                        qt[:, a, q0:q0 + cw],
                                        start=True, stop=True)
                                nc.scalar.activation(
                                    out=es[:, SOFF[j]:SOFF[j] + SW[j]],
                                    in_=st[:, 0:SW[j]],
                                    func=EXP, scale=float(D) ** -0.5)
                                nc.vector.tensor_tensor(
                                    out=es[:, SOFF[j]:SOFF[j] + 128],
                                    in0=es[:, SOFF[j]:SOFF[j] + 128],
                                    in1=mask_t[:, j % 2, :], op=MUL)

                        def v_tile(hT, jj, j):
                            pV = psS.tile([128, 1024], F32, tag="S", name="S")
                            for (n0, nw) in CCHUNKS:
                                for cb in range(CB):
                                    nc.tensor.matmul(
                                        pV[:, n0:n0 + nw], hT[:, cb, jj, :],
                                        wv_s[:, cb, n0:n0 + nw],
                                        start=(cb == 0), stop=(cb == CB - 1))
                            nc.vector.tensor_copy(
                                out=VA[j][:, :, 0:D],
                                in_=pV[:, 0:C].rearrange("p (h d) -> p h d",
                                                         d=D))

                        # own tiles first: Q^T (wq shares the pAw buffer)
                        with (
                            tc.tile_pool(name="pHo", bufs=2) as pHo,
                        ):
                            wq_s = pAw.tile([128, CB, CB, 128], BF16,
                                            tag="wk", name="wq")
                            nc.sync.dma_start(out=wq_s[:], in_=wq[:])
                            for q in range(2):
                                hTo = pHo.tile([128, CB, 4, 128], BF16,
                                               tag="hTo", name="hTo")
                                for i in range(4):
                                    ln_tile(x_own[4 * q + i],
                                            hTo[:, :, i, :])
                                qk_quad(wq_s, hTo, 4, (QTe, QTo),
                                        4 * q * 128)

                        wk_s = pAw.tile([128, CB, CB, 128], BF16, tag="wk",
                                        name="wk")
                        nc.sync.dma_start(out=wk_s[:], in_=wk[:])
                        for hh in range(2):
                            expS[hh] = pB.tile([128, STOT], BF16, tag="eS",
                                               name=f"eS{hh}")
                        for q in range(4):
                            jq = list(range(4 * q, 4 * q + 4))
                            HTq = pHT.tile([128, CB, 4, 128], BF16,
                                           tag="HTq", name="HTq")
                            for jj in range(4):
                                ln_tile(x_ctx[jq[jj]], HTq[:, :, jj, :])
                            qk_quad(wk_s, HTq, 4, (KT,), 4 * q * 128)
                            for jj in range(4):
                                v_tile(HTq, jj, jq[jj])
                            s_part(0, jq)
                            s_part(1, jq)

                    for cb in range(CB):
                        nc.sync.dma_start(out=wot[cb][:], in_=wo[cb])

                    # remaining heads: software-pipelined S/attV
                    with (
                        tc.tile_pool(name="psS2", bufs=3, space="PSUM") as psS2,
                        tc.tile_pool(name="psA", bufs=2, space="PSUM") as psA,
                    ):
                        def s_head(h):
                            a = h // 2
                            qt = QTe if h % 2 == 0 else QTo
                            es = pB.tile([128, STOT], BF16, tag="eS",
                                         name=f"eS{h}")
                            expS[h] = es
                            for j in range(NT):
                                st = psS2.tile([128, 1024], F32, tag="S",
                                               name="S")
                                for (c0, cw) in _schunks(SW[j]):
                                    q0 = (j // 2) * 128 + c0
                                    nc.tensor.matmul(
                                        st[:, c0:c0 + cw],
                                        KT[:, a, j * 128:(j + 1) * 128],
                                        qt[:, a, q0:q0 + cw],
                                        start=True, stop=True)
                                nc.scalar.activation(
                                    out=es[:, SOFF[j]:SOFF[j] + SW[j]],
                                    in_=st[:, 0:SW[j]],
                                    func=EXP, scale=float(D) ** -0.5)
                                nc.vector.tensor_tensor(
                                    out=es[:, SOFF[j]:SOFF[j] + 128],
                                    in0=es[:, SOFF[j]:SOFF[j] + 128],
                                    in1=mask_t[:, j % 2, :], op=MUL)

                        def av_head(h):
                            a, z = h // 2, (h % 2) * D
                            for k in range(2):
                                at = psA.tile([128, 512], F32, tag="attn",
                                              name="attn")
                                js = range(8) if k == 0 else range(NT)
                                for j in js:
                                    i0 = j // 2
                                    lo = max(i0, 4 * k)
                                    ps = (lo - 4 * k) * 128
                                    w = (4 * k + 4 - lo) * 128
                                    rs = (lo - i0) * 128
                                    nc.tensor.matmul(
                                        at[0:96, ps:ps + w], VA[j][:, h, :],
                                        expS[h][:, SOFF[j] + rs:
                                                SOFF[j] + rs + w],
                                        start=(j == js[0]),
                                        stop=(j == js[-1]))
                                den = normp.tile([32, 512], F32, tag="den",
                                                 name="den")
                                nc.vector.tensor_copy(out=den[:],
                                                      in_=at[D:D + 32, :])
                                rcp = normp.tile([32, 512], F32, tag="rcp",
                                                 name="rcp")
                                nc.vector.reciprocal_approx_fast(
                                    out=rcp[:], in_=den[:])
                                ks = slice(k * 512, (k + 1) * 512)
                                nc.vector.tensor_tensor(
                                    out=ATT[a][z:z + 32, ks],
                                    in0=at[0:32, :], in1=rcp[:], op=MUL)
                                nc.vector.tensor_tensor(
                                    out=ATT[a][z + 32:z + D, ks],
                                    in0=at[32:D, :], in1=rcp[:], op=MUL)

                        av_head(0)
                        for h in range(2, H):
                            s_head(h)
                            av_head(h - 1)
                        av_head(H - 1)

            # ---- Phase C: Wo + residual; LN2 + MLP + residual ---------------
            with tc.tile_pool(name="p3w", bufs=1) as p3w:
                X2 = [p3w.tile([128, C], F32, tag=f"X2{i}", name=f"X2{i}")
                      for i in range(NS)]
                xbuf = [p3w.tile([128, C], F32, tag=f"xb{i}", name=f"xb{i}")
                        for i in range(NS)]
                for i in range(NS):
                    nc.sync.dma_start(out=xbuf[i][:], in_=x_own[i])
                W2S = p3w.tile([128, MB, C], BF16, tag="W2S", name="W2S")
                for m in range(MB):
                    nc.gpsimd.dma_start(out=W2S[:, m, :], in_=w2[m])

                h2T = [p3w.tile([128, NS * 128], BF16, tag=f"h2T{cb}",
                                name=f"h2T{cb}") for cb in range(CB)]
                hidT = [p3w.tile([128, NS * 128], BF16, tag=f"hid{m}",
                                 name=f"hid{m}") for m in range(MB)]

                with (
                    tc.tile_pool(name="pC", bufs=3) as pC,
                    tc.tile_pool(name="psW", bufs=2, space="PSUM") as psW,
                    tc.tile_pool(name="psT2", bufs=2, space="PSUM") as psT2,
                    tc.tile_pool(name="psM", bufs=2, space="PSUM") as psM,
                ):
                    h2buf = [None] * NS

                    def wo_slot(i):
                        """Wo matmul + residual + LN2; h2 kept for transposes."""
                        xt = xbuf[i]
                        pt = psW.tile([128, C], F32, tag="wops", name="wops")
                        for (n0, nw) in CCHUNKS:
                            for a in range(HP):
                                nc.tensor.matmul(
                                    pt[:, n0:n0 + nw],
                                    ATT[a][:, i * 128:(i + 1) * 128],
                                    wot[a][:, n0:n0 + nw],
                                    start=(a == 0), stop=(a == HP - 1))
                        nc.vector.tensor_tensor(
                            out=X2[i][:], in0=pt[:], in1=xt[:], op=ADD)
                        rstd, negms = _ln_stats(nc, small, X2[i], eps_t)
                        h2 = pC.tile([128, C], BF16, tag="h2", name=f"h2_{i}")
                        nc.scalar.activation(out=h2[:], in_=X2[i][:],
                                             func=IDENT, bias=negms[:],
                                             scale=rstd[:])
                        h2buf[i] = h2

                    def h2_tr(i):
                        pt = psT2.tile([128, CB, 128], BF16, tag="tr2",
                                       name="tr2")
                        for cb in range(CB):
                            nc.tensor.transpose(
                                pt[:, cb, :],
                                h2buf[i][:, cb * 128:(cb + 1) * 128], identb[:])
                        for cb in range(CB):
                            nc.vector.tensor_copy(
                                out=h2T[cb][:, i * 128:(i + 1) * 128],
                                in_=pt[:, cb, :])

                    def mlp1(m):
                        w1t = pC.tile([128, CB, 128], BF16, tag="w1t",
                                      name="w1t")
                        nc.sync.dma_start(out=w1t[:], in_=w1[m])
                        for sc in range(2):
                            pt = psM.tile([128, 512], F32, tag="mlp1",
                                          name="mlp1")
                            for cb in range(CB):
                                nc.tensor.matmul(
                                    pt[:], w1t[:, cb, :],
                                    h2T[cb][:, sc * 512:(sc + 1) * 512],
                                    start=(cb == 0), stop=(cb == CB - 1))
                            nc.scalar.activation(
                                out=hidT[m][:, sc * 512:(sc + 1) * 512],
                                in_=pt[:], func=GELU)

                    wo_slot(0)
                    wo_slot(1)
                    for i in range(2, NS):
                        wo_slot(i)
                        h2_tr(i - 2)
                    h2_tr(NS - 2)
                    h2_tr(NS - 1)
                    for m in range(MB):
                        mlp1(m)

                with (
                    tc.tile_pool(name="pD", bufs=2) as pD,
                    tc.tile_pool(name="psY", bufs=2, space="PSUM") as psY,
                ):
                    for i in range(NS):
                        yt = pD.tile([128, C], F32, tag="yt", name="yt")
                        pt = psY.tile([128, C], F32, tag="mlp2", name="mlp2")
                        for (n0, nw) in CCHUNKS:
                            for m in range(MB):
                                nc.tensor.matmul(
                                    pt[:, n0:n0 + nw],
                                    hidT[m][:, i * 128:(i + 1) * 128],
                                    W2S[:, m, n0:n0 + nw],
                                    start=(m == 0), stop=(m == MB - 1))
                        nc.vector.tensor_tensor(
                            out=yt[:], in0=pt[:], in1=X2[i][:], op=ADD)
                        nc.sync.dma_start(out=y[i], in_=yt[:])

    nc.finalize()
    return nc


_NC = None
LAST_RESULTS = None


def _get_program():
    global _NC
    if _NC is None:
        _NC = build_program()
    return _NC


def _core_inputs(inputs):
    x = np.ascontiguousarray(np.asarray(inputs["x"], np.float32))
    wq_f = np.transpose(np.asarray(inputs["Wq"], np.float32), (1, 0, 2)
                        ).reshape(C, C)
    wk_f = np.transpose(np.asarray(inputs["Wk"], np.float32), (1, 0, 2)
                        ).reshape(C, C)
    wv_f = np.transpose(np.asarray(inputs["Wv"], np.float32), (1, 0, 2)
                        ).reshape(C, C)
    # d-major: [c-part, c-chunk, d-chunk, 128]
    wq = np.ascontiguousarray(
        wq_f.reshape(CB, 128, CB, 128).transpose(1, 0, 2, 3)
    ).astype(ml_dtypes.bfloat16)
    wk = np.ascontiguousarray(
        wk_f.reshape(CB, 128, CB, 128).transpose(1, 0, 2, 3)
    ).astype(ml_dtypes.bfloat16)
    # token-major: [c-part, c-chunk, 768]
    wv = np.ascontiguousarray(
        wv_f.reshape(CB, 128, C).transpose(1, 0, 2)).astype(ml_dtypes.bfloat16)
    wo = np.asarray(inputs["Wo"], np.float32).reshape(CB, 128, C).astype(
        ml_dtypes.bfloat16)
    w1 = np.ascontiguousarray(
        np.asarray(inputs["W1"], np.float32).reshape(CB, 128, MB, 128)
        .transpose(2, 1, 0, 3)).astype(ml_dtypes.bfloat16)
    w2 = np.asarray(inputs["W2"], np.float32).reshape(MB, 128, C).astype(
        ml_dtypes.bfloat16)

    tri = (np.arange(128)[:, None] <= np.arange(128)[None, :]).astype(np.float32)
    masks = {
        0: np.stack([tri, np.zeros((128, 128), np.float32)], axis=1),
        1: np.stack([np.ones((128, 128), np.float32), tri], axis=1),
    }
    in_maps = []
    for core in range(8):
        b, p = core // 2, core % 2
        own = [2 * i + p for i in range(NS)]
        x_b = x[b].reshape(NT, 128, C)
        in_maps.append({
            "x_ctx": x_b,
            "x_own": np.ascontiguousarray(x_b[own]),
            "wq": wq, "wk": wk, "wv": wv, "wo": wo, "w1": w1, "w2": w2,
            "mask": np.ascontiguousarray(masks[p]).astype(ml_dtypes.bfloat16),
        })
    return in_maps


def kernel(**inputs):
    global LAST_RESULTS
    nc = _get_program()
    in_maps = _core_inputs(inputs)
    trace = bool(int(os.environ.get("KERNEL_TRACE", "0")))
    res = run_bass_kernel_spmd(
        nc, in_maps, core_ids=list(range(8)), trace=trace,
        trace_cores=list(range(8)) if trace else None,
    )
    LAST_RESULTS = res
    out = np.empty((B, T, C), np.float32)
    for core in range(8):
        b, p = core // 2, core % 2
        yc = res.results[core]["y"]  # [8, 128, 768]
        for i in range(NS):
            g = 2 * i + p
            out[b, g * 128:(g + 1) * 128, :] = yc[i]
    return out
